# revision 32
# baseline (speedup 1.0000x reference)
"""Trainium2 Bass kernel for MultiHeadEdgeAwareMessagePassing.

Math restructure (validated vs reference):
  logits[i,j,h] = s_q[i,h] + s_k[j,h] + w[i,j]*c1[h] + c0[h]   (valid j: w>0)
  alpha = softmax_j(logits) * w
s_q, c0 cancel in the softmax; bk's contribution to s_k cancels too. With
g[j,h] = exp(h[j]@a_k[h]), a_k[h] = Wk[h-block]^T u_k[h], v = h@Wv^T:
  msg[i,h,:] = Num_h[i,:] / Den_h[i]
  Num_h = W1^T (g_h*v_h)   (+ (W1^T g_h)*bv_h if bv != 0)
  Den_h = mask^T g_h + c1_h (W1^T g_h)
where mask=[w>0], W1=relu(w)  (exp(c1 w) ~= 1 + c1 w; |c1 w| << 1).

Sharding: destination rows i split across 8 cores (384 rows each). Each core
reads its relu(w)^T slice (fp8, host-pretiled for contiguous DMA), replicated
h^T (fp8) and small weights (bf16/f32). mask is recomputed on device from the
fp8 W1 via Sign on the scalar engine.

Schedule: software-pipelined by one chunk — the PE runs chunk ch-1's psA
matmuls while scalar/DVE produce chunk ch's rhs (v*g), so the PE stream stays
dense and HAM-warm. Epilogue is batched across the 3 i-subtiles: one psA PSUM
tile [128,3,512], residual-add via identity matmul, mean/var via scalar
accum_out, single fused output DMA.
"""

import numpy as np

N = 3072
D = 256
H = 4
DH = 64
DE = 8
NCORES = 8
ISLICE = N // NCORES       # 384
NSUB = ISLICE // 128       # 3
CJT = 4                    # j-tiles (of 128) per chunk
NCH = N // (128 * CJT)     # 6 chunks of 512 j
CHW = CJT * ISLICE         # 1536 wt cols per chunk

_cache = {}


def _build_bass(flags):
    import concourse.bass as bass
    import concourse.tile as tile
    from concourse import bacc, mybir
    from concourse.bass import ts

    dt = mybir.dt
    AF = mybir.ActivationFunctionType
    OP = mybir.AluOpType

    nc = bacc.Bacc("TRN2", target_bir_lowering=False, debug=False,
                   num_devices=NCORES)

    f8 = dt.float8e4
    bf = dt.bfloat16
    f32 = dt.float32

    has_bv, has_gb = flags
    # host-pretiled, per chunk 3072 cols: first 1536 = relu(w)^T tiles,
    # next 1536 = mask tiles (same [jm, s, ii] layout)
    wt_d = nc.dram_tensor("wt", [128, NCH * 2 * CHW], f8,
                          kind="ExternalInput")
    # ht[p, a, j] = h[j, a*128+p]
    ht_d = nc.dram_tensor("ht", [128, 2, N], f8, kind="ExternalInput")
    # su1[p, a, 0:256] = Wv^T block a ; su1[p, a, 256:260] = a_k block a
    su1_d = nc.dram_tensor("su1", [128, 2, 260], bf, kind="ExternalInput")
    # su2 bf16: WoT 0:512 | ident 512:640 | hs 640:1408 | c1 1408:1412
    #           | gamma 1412:1668 | beta 1668:1924 | bv 1924:2180
    su2_d = nc.dram_tensor("su2", [128, 2180], bf, kind="ExternalInput")
    # out[p, s, d] = result row (i0 + s*128 + p)
    out_d = nc.dram_tensor("out", [128, NSUB, D], f32, kind="ExternalOutput")

    with tile.TileContext(nc) as tc:
        with (
            tc.tile_pool(name="consts", bufs=1) as consts,
            tc.tile_pool(name="wtp", bufs=NCH) as wtp,
            tc.tile_pool(name="rhsp", bufs=4) as rhsp,
            tc.tile_pool(name="small", bufs=8) as small,
            tc.tile_pool(name="outp", bufs=3) as outp,
            tc.tile_pool(name="acc", bufs=1, space="PSUM") as accp,
            tc.tile_pool(name="pvk", bufs=2, space="PSUM") as pvk,
            tc.tile_pool(name="warmp", bufs=1, space="PSUM") as warmp,
        ):
            # ---- setup consts ----
            su1 = consts.tile([128, 2, 260], bf, tag="su1")
            eps_sb = consts.tile([128, 1], f32, tag="eps")
            nc.vector.memset(eps_sb, 1e-5)

            # PE warm-up: dummy matmuls during the DMA ramp keep the HAM
            # activity window busy so real matmuls start at full clock.
            # The Sqrt feeding warm_src also forces the sqrt ACT table to
            # load now (idle ramp) instead of on the epilogue critical path.
            warm_src = consts.tile([128, 128], bf, tag="warm")
            nc.vector.memset(warm_src, 0.0)
            nc.scalar.activation(warm_src[:, 0:1], eps_sb, AF.Sqrt)
            warm_ps = warmp.tile([128, 128], f32, tag="wps")
            for _ in range(30):
                nc.tensor.matmul(warm_ps, warm_src, warm_src,
                                 start=True, stop=True,
                                 skip_group_check=True)

            # persistent accumulator, one bank per i-subtile:
            # [:, s, 0:256] = W1.gV, 256:260 = W1.g, 260:264 = mask.g
            psA = accp.tile([128, NSUB, 512], f32, tag="A", name="psA")

            ht_sb = consts.tile([128, 2, N], f8, tag="ht")
            wt_tiles = [wtp.tile([128, 2 * CHW], f8, tag="wt", name=f"wt_{ch}")
                        for ch in range(NCH)]
            # ht in two halves + one wm DMA per chunk; first compute needs
            # only ht half 0 + su1, then wm0
            nc.sync.dma_start(ht_sb[:, :, 0:N // 2],
                              ht_d.ap()[:, :, 0:N // 2])
            nc.sync.dma_start(su1, su1_d.ap())
            for ch in range(NCH):
                nc.sync.dma_start(wt_tiles[ch], wt_d.ap()[:, ts(ch, 2 * CHW)])
                if ch == 1:
                    nc.sync.dma_start(ht_sb[:, :, N // 2:N],
                                      ht_d.ap()[:, :, N // 2:N])

            # ---------------- main loop (pipelined by one chunk) ---------
            rhs_tiles = {}

            def produce(ch):
                rhs_pair = []
                for hc in range(2):  # half-chunks of 2 j-tiles
                    ps_vk = pvk.tile([128, 2, 512], f32, tag="vk")
                    for jl in range(2):
                        jt = ch * CJT + hc * 2 + jl
                        for a in range(2):
                            nc.tensor.matmul(ps_vk[:, jl, 0:260],
                                             ht_sb[:, a, ts(jt, 128)],
                                             su1[:, a, :],
                                             start=(a == 0), stop=(a == 1))
                    rhs4 = rhsp.tile([128, 2, 260], bf, tag="rhs4",
                                     name=f"rhs{ch}_{hc}")
                    # g = exp(s_k) straight into the 4 tail cols of rhs4
                    nc.scalar.activation(rhs4[:, :, 256:260],
                                         ps_vk[:, :, 256:260], AF.Exp)
                    # rhs4[:, :, 0:256] = v * g (g broadcast over DH)
                    gv = rhs4[:, :, 256:260]
                    gb = bass.AP(tensor=gv.tensor, offset=gv.offset,
                                 ap=[gv.ap[0], gv.ap[1], gv.ap[2], [0, DH]])
                    nc.vector.tensor_tensor(
                        out=rhs4[:, :, 0:256].rearrange(
                            "p j (h d) -> p j h d", h=H),
                        in0=ps_vk[:, :, 0:256].rearrange(
                            "p j (h d) -> p j h d", h=H),
                        in1=gb, op=OP.mult)
                    rhs_pair.append(rhs4)
                rhs_tiles[ch] = rhs_pair

            def consume(ch):
                wt4 = wt_tiles[ch]
                rhs_pair = rhs_tiles.pop(ch)
                for hc in range(2):
                    rhs4 = rhs_pair[hc]
                    for jl in range(2):
                        off = (hc * 2 + jl) * ISLICE
                        st = (ch == 0 and hc == 0 and jl == 0)
                        sp = (ch == NCH - 1 and hc == 1 and jl == 1)
                        for s in range(NSUB):
                            sl = slice(off + s * 128, off + (s + 1) * 128)
                            ml = slice(CHW + off + s * 128,
                                       CHW + off + (s + 1) * 128)
                            nc.tensor.matmul(psA[:, s, 0:260], wt4[:, sl],
                                             rhs4[:, jl, :], start=st,
                                             stop=sp, skip_group_check=True)
                            nc.tensor.matmul(psA[:, s, 260:264], wt4[:, ml],
                                             rhs4[:, jl, 256:260],
                                             start=st, stop=sp,
                                             skip_group_check=True)

            produce(0)
            for ch in range(NCH):
                if ch + 1 < NCH:
                    produce(ch + 1)
                consume(ch)

            # ---------------- epilogue consts (tail of sync queue) -------
            su2 = consts.tile([128, 2180], bf, tag="su2")
            nc.sync.dma_start(su2, su2_d.ap())
            WoT_sb = su2[:, 0:512].rearrange("p (a n) -> p a n", a=2)
            ident = su2[:, 512:640]
            hs_sb = su2[:, 640:1408].rearrange("p (s n) -> p s n", s=NSUB)
            c1b = su2[:, 1408:1412]
            gam = su2[:, 1412:1668]
            bet = su2[:, 1668:1924]

            # ---------------- epilogue (batched across s) ----------------
            # den[p, s, h] = c1[h]*W1.g + mask.g ; rden = 1/den
            c1bb = bass.AP(tensor=c1b.tensor, offset=c1b.offset,
                           ap=[c1b.ap[0], [0, NSUB], c1b.ap[1]])
            den = small.tile([128, NSUB, H], f32, tag="den")
            nc.vector.tensor_tensor(out=den, in0=psA[:, :, 256:260],
                                    in1=c1bb, op=OP.mult)
            nc.vector.tensor_add(den, den, psA[:, :, 260:264])
            rden = small.tile([128, NSUB, H], f32, tag="rden")
            nc.vector.reciprocal(rden, den)

            # msg = Num * rden (rden broadcast over DH)
            numf = None
            if has_bv:
                gcol = psA[:, :, 256:260]
                gcb = bass.AP(tensor=gcol.tensor, offset=gcol.offset,
                              ap=[gcol.ap[0], gcol.ap[1], gcol.ap[2],
                                  [0, DH]])
                bvc = su2[:, 1924:2180]
                bvb = bass.AP(tensor=bvc.tensor, offset=bvc.offset,
                              ap=[bvc.ap[0], [0, NSUB], [DH, H], [1, DH]])
                numf = outp.tile([128, NSUB, D], f32, tag="numf")
                nc.vector.tensor_tensor(
                    out=numf.rearrange("p s (h d) -> p s h d", h=H),
                    in0=gcb, in1=bvb, op=OP.mult)
                nc.vector.tensor_add(numf, numf, psA[:, :, 0:256])
            # per-s pipeline: msg (DVE) -> transpose (PE) -> copy (scalar)
            # -> Wo + residual (PE) -> bn stats (DVE)
            msg = outp.tile([128, NSUB, D], bf, tag="msg")
            pst = pvk.tile([128, 2 * NSUB, 128], bf, tag="vk", name="pst")
            msgT = outp.tile([128, 2 * NSUB, 128], bf, tag="msgT")
            ps_o = accp.tile([128, NSUB, 512], f32, tag="A", name="pso")
            stats = small.tile([128, NSUB, 6], f32, tag="stats")
            mv = small.tile([128, NSUB, 2], f32, tag="mv")
            for s in range(NSUB):
                if numf is not None:
                    num_s = numf[:, s, :].rearrange("p (h d) -> p h d", h=H)
                else:
                    num_s = psA[:, s, 0:256].rearrange(
                        "p (h d) -> p h d", h=H)
                rds = rden[:, s, :]
                rdb_s = bass.AP(tensor=rds.tensor, offset=rds.offset,
                                ap=[rds.ap[0], rds.ap[1], [0, DH]])
                nc.vector.tensor_tensor(
                    out=msg[:, s, :].rearrange("p (h d) -> p h d", h=H),
                    in0=num_s, in1=rdb_s, op=OP.mult)
                for b in range(2):
                    nc.tensor.transpose(pst[:, 2 * s + b, :],
                                        msg[:, s, ts(b, 128)], ident)
                nc.scalar.activation(msgT[:, 2 * s:2 * s + 2, :],
                                     pst[:, 2 * s:2 * s + 2, :], AF.Copy)
                nc.tensor.matmul(ps_o[:, s, 0:256], msgT[:, 2 * s, :],
                                 WoT_sb[:, 0, :], start=True, stop=False)
                nc.tensor.matmul(ps_o[:, s, 0:256], msgT[:, 2 * s + 1, :],
                                 WoT_sb[:, 1, :], start=False, stop=False)
                # x = msg@WoT + (h + bo): residual added on the PE
                nc.tensor.matmul(ps_o[:, s, 0:256], ident, hs_sb[:, s, :],
                                 start=False, stop=True)
                nc.vector.bn_stats(out=stats[:, s, :], in_=ps_o[:, s, 0:256])
                nc.vector.bn_aggr(out=mv[:, s, :], in_=stats[:, s, :])

            sd = small.tile([128, NSUB], f32, tag="sd")
            nc.scalar.activation(sd, mv[:, :, 1], AF.Sqrt, bias=eps_sb)
            rstd = small.tile([128, NSUB], f32, tag="rstd")
            nc.vector.reciprocal(rstd, sd)
            # y = x*rstd - mean*rstd, done on the scalar engine as
            # Identity(x*scale + bias) with per-partition scale/bias APs
            nmr = small.tile([128, NSUB], f32, tag="nmr")
            nc.vector.tensor_tensor(out=nmr, in0=mv[:, :, 0], in1=rstd,
                                    op=OP.mult)
            nc.vector.tensor_scalar(nmr, nmr, -1.0, None, op0=OP.mult)

            ot = outp.tile([128, NSUB, D], f32, tag="ot")
            if has_gb:
                y = outp.tile([128, NSUB, D], bf, tag="y")
                for s in range(NSUB):
                    nc.scalar.activation(y[:, s, :], ps_o[:, s, 0:256],
                                         AF.Identity, bias=nmr[:, s:s + 1],
                                         scale=rstd[:, s:s + 1])
                gamb = bass.AP(tensor=gam.tensor, offset=gam.offset,
                               ap=[gam.ap[0], [0, NSUB], gam.ap[1]])
                betb = bass.AP(tensor=bet.tensor, offset=bet.offset,
                               ap=[bet.ap[0], [0, NSUB], bet.ap[1]])
                yg = outp.tile([128, NSUB, D], bf, tag="yg")
                nc.vector.tensor_tensor(out=yg, in0=y, in1=gamb, op=OP.mult)
                nc.vector.tensor_tensor(out=ot, in0=yg, in1=betb, op=OP.add)
                nc.sync.dma_start(out_d.ap(), ot)
            else:
                # gamma==1, beta==0: normalized x is the final output
                for s in range(NSUB):
                    nc.scalar.activation(ot[:, s, :], ps_o[:, s, 0:256],
                                         AF.Identity, bias=nmr[:, s:s + 1],
                                         scale=rstd[:, s:s + 1])
                    nc.sync.dma_start(out_d.ap()[:, s, :], ot[:, s, :])

    nc.compile()
    return nc


def _make_in_maps(h, w, Wk, Wv, bv, We_w, u, Wo, bo, gamma, beta, **_unused):
    import ml_dtypes
    f = np.float32
    b16 = ml_dtypes.bfloat16
    e4 = ml_dtypes.float8_e4m3
    h = np.asarray(h, dtype=f)
    w = np.asarray(w, dtype=f)
    Wk = np.asarray(Wk, dtype=f)
    u = np.asarray(u, dtype=f)
    We_w = np.asarray(We_w, dtype=f)
    bv = np.asarray(bv, dtype=f)
    gamma_f = np.asarray(gamma, dtype=f)
    beta_f = np.asarray(beta, dtype=f)
    has_bv = bool(np.any(bv != 0))
    has_gb = bool(np.any(gamma_f != 1.0) or np.any(beta_f != 0.0))

    # ht[p, a, j] = h[j, a*128+p]
    ht = np.ascontiguousarray(
        h.T.reshape(2, 128, N).transpose(1, 0, 2)).astype(e4)

    # su1: Wv^T blocks + a_k blocks
    su1 = np.zeros((128, 2, 260), f)
    for a in range(2):
        su1[:, a, 0:256] = np.asarray(Wv, dtype=f)[:, a * 128:(a + 1) * 128].T
    ak = np.einsum('hdc,hd->ch', Wk.reshape(H, DH, D), u[:, DH:2 * DH])
    su1[:, 0, 256:260] = ak[0:128, :]
    su1[:, 1, 256:260] = ak[128:256, :]

    # su2: WoT | identity | hs (per core) | c1 | gamma | beta | bv
    c1 = np.einsum('hd,hd->h', We_w[:, 0].reshape(H, DE),
                   u[:, 2 * DH:2 * DH + DE])
    su2_base = np.zeros((128, 2180), f)
    WoT = np.asarray(Wo, dtype=f).T
    su2_base[:, 0:512] = WoT.reshape(2, 128, D).transpose(
        1, 0, 2).reshape(128, 512)
    su2_base[:, 512:640] = np.eye(128, dtype=f)
    su2_base[:, 1408:1412] = c1[None, :]
    su2_base[:, 1412:1668] = gamma_f[None, :]
    su2_base[:, 1668:1924] = beta_f[None, :]
    if has_bv:
        su2_base[:, 1924:2180] = bv[None, :]

    bo_f = np.asarray(bo, dtype=f)
    wT_relu = np.maximum(w.T, 0.0)

    common = {
        "ht": ht,
        "su1": su1.astype(b16),
    }
    in_maps = []
    for c in range(NCORES):
        sl = slice(c * ISLICE, (c + 1) * ISLICE)
        m = dict(common)
        # wt[p, ch, 0, jm, ii] = relu(w)[i0+ii, (ch*4+jm)*128+p]
        # wt[p, ch, 1, jm, ii] = mask
        wtc = wT_relu[:, sl].reshape(NCH, CJT, 128, ISLICE)
        wtc = wtc.transpose(2, 0, 1, 3).reshape(128, NCH, 1, CHW)
        mskc = (wtc > 0).astype(f)
        m["wt"] = np.ascontiguousarray(
            np.concatenate([wtc, mskc], axis=2).reshape(
                128, NCH * 2 * CHW)).astype(e4)
        su2 = su2_base.copy()
        su2[:, 640:1408] = (h[sl, :] + bo_f[None, :]).reshape(
            NSUB, 128, D).transpose(1, 0, 2).reshape(128, 768)
        m["su2"] = su2.astype(b16)
        in_maps.append(m)
    return in_maps, (has_bv, has_gb)


def kernel(**inputs):
    from concourse.bass_utils import run_bass_kernel_spmd

    in_maps, flags = _make_in_maps(**inputs)
    key = ("nc",) + flags
    if key not in _cache:
        _cache[key] = _build_bass(flags)
    nc = _cache[key]

    res = run_bass_kernel_spmd(nc, in_maps, core_ids=list(range(NCORES)))
    parts = [np.asarray(r["out"]).transpose(1, 0, 2).reshape(ISLICE, D)
             for r in res.results]
    out = np.concatenate(parts, axis=0)
    return np.ascontiguousarray(out, dtype=np.float32)


# revision 35
# speedup vs baseline: 1.1597x; 1.1597x over previous
"""Trainium2 Bass kernel for MultiHeadEdgeAwareMessagePassing.

Math restructure (validated vs reference):
  logits[i,j,h] = s_q[i,h] + s_k[j,h] + w[i,j]*c1[h] + c0[h]   (valid j: w>0)
  alpha = softmax_j(logits) * w
s_q, c0 cancel in the softmax; bk's contribution to s_k cancels too. With
g[j,h] = exp(h[j]@a_k[h]), a_k[h] = Wk[h-block]^T u_k[h], v = h@Wv^T:
  msg[i,h,:] = Num_h[i,:] / Den_h[i]
  Num_h = W1^T (g_h*v_h)   (+ (W1^T g_h)*bv_h if bv != 0)
  Den_h = mask^T g_h + c1_h (W1^T g_h)
where mask=[w>0], W1=relu(w)  (exp(c1 w) ~= 1 + c1 w; |c1 w| << 1).

Sharding: destination rows i split across 8 cores (384 rows each). Each core
reads its relu(w)^T slice (fp8, host-pretiled for contiguous DMA), replicated
h^T (fp8) and small weights (bf16/f32). mask is recomputed on device from the
fp8 W1 via Sign on the scalar engine.

Schedule: software-pipelined by one chunk — the PE runs chunk ch-1's psA
matmuls while scalar/DVE produce chunk ch's rhs (v*g), so the PE stream stays
dense and HAM-warm. Epilogue is batched across the 3 i-subtiles: one psA PSUM
tile [128,3,512], residual-add via identity matmul, mean/var via scalar
accum_out, single fused output DMA.
"""

import numpy as np

N = 3072
D = 256
H = 4
DH = 64
DE = 8
NCORES = 8
ISLICE = N // NCORES       # 384
NSUB = ISLICE // 128       # 3
CJT = 4                    # j-tiles (of 128) per chunk
NCH = N // (128 * CJT)     # 6 chunks of 512 j
CHW = CJT * ISLICE         # 1536 wt cols per chunk

_cache = {}


def _build_bass(flags):
    import concourse.bass as bass
    import concourse.tile as tile
    from concourse import bacc, mybir
    from concourse.bass import ts

    dt = mybir.dt
    AF = mybir.ActivationFunctionType
    OP = mybir.AluOpType

    nc = bacc.Bacc("TRN2", target_bir_lowering=False, debug=False,
                   num_devices=NCORES)

    f8 = dt.float8e4
    bf = dt.bfloat16
    f32 = dt.float32

    has_bv, has_gb = flags
    # host-pretiled, per chunk 3072 cols: first 1536 = relu(w)^T tiles,
    # next 1536 = mask tiles (same [jm, s, ii] layout)
    wt_d = nc.dram_tensor("wt", [128, NCH * 2 * CHW], f8,
                          kind="ExternalInput")
    # ht[p, a, j] = h[j, a*128+p]
    ht_d = nc.dram_tensor("ht", [128, 2, N], f8, kind="ExternalInput")
    # su1[p, a, 0:256] = Wv^T block a ; su1[p, a, 256:260] = a_k block a
    su1_d = nc.dram_tensor("su1", [128, 2, 260], bf, kind="ExternalInput")
    # su2 bf16: WoT 0:512 | ident 512:640 | hs 640:1408 | c1 1408:1412
    #           | gamma 1412:1668 | beta 1668:1924 | bv 1924:2180
    su2_d = nc.dram_tensor("su2", [128, 2180], bf, kind="ExternalInput")
    # out[p, s, d] = result row (i0 + s*128 + p)
    out_d = nc.dram_tensor("out", [128, NSUB, D], f32, kind="ExternalOutput")

    with tile.TileContext(nc) as tc:
        with (
            tc.tile_pool(name="consts", bufs=1) as consts,
            tc.tile_pool(name="wtp", bufs=NCH) as wtp,
            tc.tile_pool(name="rhsp", bufs=4) as rhsp,
            tc.tile_pool(name="small", bufs=8) as small,
            tc.tile_pool(name="outp", bufs=3) as outp,
            tc.tile_pool(name="acc", bufs=1, space="PSUM") as accp,
            tc.tile_pool(name="pvk", bufs=2, space="PSUM") as pvk,
            tc.tile_pool(name="warmp", bufs=1, space="PSUM") as warmp,
        ):
            # ---- setup consts ----
            su1 = consts.tile([128, 2, 260], bf, tag="su1")
            eps_sb = consts.tile([128, 1], f32, tag="eps")
            nc.vector.memset(eps_sb, 1e-5)

            # PE warm-up: dummy matmuls during the DMA ramp keep the HAM
            # activity window busy so real matmuls start at full clock
            warm_src = consts.tile([128, 128], bf, tag="warm")
            nc.vector.memset(warm_src, 0.0)
            warm_ps = warmp.tile([128, 128], f32, tag="wps")
            for _ in range(30):
                nc.tensor.matmul(warm_ps, warm_src, warm_src,
                                 start=True, stop=True,
                                 skip_group_check=True)

            # persistent accumulator, one bank per i-subtile:
            # [:, s, 0:256] = W1.gV, 256:260 = W1.g, 260:264 = mask.g
            psA = accp.tile([128, NSUB, 512], f32, tag="A", name="psA")

            ht_sb = consts.tile([128, 2, N], f8, tag="ht")
            wt_tiles = [wtp.tile([128, 2 * CHW], f8, tag="wt", name=f"wt_{ch}")
                        for ch in range(NCH)]
            # ht in two halves + one wm DMA per chunk; first compute needs
            # only ht half 0 + su1, then wm0
            nc.sync.dma_start(ht_sb[:, :, 0:N // 2],
                              ht_d.ap()[:, :, 0:N // 2])
            nc.sync.dma_start(su1, su1_d.ap())
            for ch in range(NCH):
                nc.sync.dma_start(wt_tiles[ch], wt_d.ap()[:, ts(ch, 2 * CHW)])
                if ch == 1:
                    nc.sync.dma_start(ht_sb[:, :, N // 2:N],
                                      ht_d.ap()[:, :, N // 2:N])

            # ---------------- main loop (pipelined by one chunk) ---------
            rhs_tiles = {}

            def produce(ch):
                rhs_pair = []
                for hc in range(2):  # half-chunks of 2 j-tiles
                    ps_vk = pvk.tile([128, 2, 512], f32, tag="vk")
                    for jl in range(2):
                        jt = ch * CJT + hc * 2 + jl
                        for a in range(2):
                            nc.tensor.matmul(ps_vk[:, jl, 0:260],
                                             ht_sb[:, a, ts(jt, 128)],
                                             su1[:, a, :],
                                             start=(a == 0), stop=(a == 1))
                    rhs4 = rhsp.tile([128, 2, 260], bf, tag="rhs4",
                                     name=f"rhs{ch}_{hc}")
                    # g = exp(s_k) straight into the 4 tail cols of rhs4
                    nc.scalar.activation(rhs4[:, :, 256:260],
                                         ps_vk[:, :, 256:260], AF.Exp)
                    # rhs4[:, :, 0:256] = v * g (g broadcast over DH)
                    gv = rhs4[:, :, 256:260]
                    gb = bass.AP(tensor=gv.tensor, offset=gv.offset,
                                 ap=[gv.ap[0], gv.ap[1], gv.ap[2], [0, DH]])
                    nc.vector.tensor_tensor(
                        out=rhs4[:, :, 0:256].rearrange(
                            "p j (h d) -> p j h d", h=H),
                        in0=ps_vk[:, :, 0:256].rearrange(
                            "p j (h d) -> p j h d", h=H),
                        in1=gb, op=OP.mult)
                    rhs_pair.append(rhs4)
                rhs_tiles[ch] = rhs_pair

            def consume(ch):
                wt4 = wt_tiles[ch]
                rhs_pair = rhs_tiles.pop(ch)
                for hc in range(2):
                    rhs4 = rhs_pair[hc]
                    for jl in range(2):
                        off = (hc * 2 + jl) * ISLICE
                        st = (ch == 0 and hc == 0 and jl == 0)
                        sp = (ch == NCH - 1 and hc == 1 and jl == 1)
                        for s in range(NSUB):
                            sl = slice(off + s * 128, off + (s + 1) * 128)
                            ml = slice(CHW + off + s * 128,
                                       CHW + off + (s + 1) * 128)
                            nc.tensor.matmul(psA[:, s, 0:260], wt4[:, sl],
                                             rhs4[:, jl, :], start=st,
                                             stop=sp, skip_group_check=True)
                            nc.tensor.matmul(psA[:, s, 260:264], wt4[:, ml],
                                             rhs4[:, jl, 256:260],
                                             start=st, stop=sp,
                                             skip_group_check=True)

            produce(0)
            # dummy Sqrt early in the scalar stream: its ACT_TABLE_LOAD
            # lands in main-loop scalar slack, not the epilogue chain
            junk1 = small.tile([128, 1], f32, tag="junk1")
            nc.scalar.activation(junk1, eps_sb, AF.Sqrt)
            for ch in range(NCH):
                if ch + 1 < NCH:
                    produce(ch + 1)
                consume(ch)

            # ---------------- epilogue consts (tail of sync queue) -------
            su2 = consts.tile([128, 2180], bf, tag="su2")
            nc.sync.dma_start(su2, su2_d.ap())
            WoT_sb = su2[:, 0:512].rearrange("p (a n) -> p a n", a=2)
            ident = su2[:, 512:640]
            hs_sb = su2[:, 640:1408].rearrange("p (s n) -> p s n", s=NSUB)
            c1b = su2[:, 1408:1412]
            gam = su2[:, 1412:1668]
            bet = su2[:, 1668:1924]

            # ---------------- epilogue (batched across s) ----------------
            # den[p, s, h] = c1[h]*W1.g + mask.g ; rden = 1/den
            c1bb = bass.AP(tensor=c1b.tensor, offset=c1b.offset,
                           ap=[c1b.ap[0], [0, NSUB], c1b.ap[1]])
            den = small.tile([128, NSUB, H], f32, tag="den")
            nc.vector.tensor_tensor(out=den, in0=psA[:, :, 256:260],
                                    in1=c1bb, op=OP.mult)
            nc.vector.tensor_add(den, den, psA[:, :, 260:264])
            rden = small.tile([128, NSUB, H], f32, tag="rden")
            nc.vector.reciprocal(rden, den)

            # msg = Num * rden (rden broadcast over DH)
            numf = None
            if has_bv:
                gcol = psA[:, :, 256:260]
                gcb = bass.AP(tensor=gcol.tensor, offset=gcol.offset,
                              ap=[gcol.ap[0], gcol.ap[1], gcol.ap[2],
                                  [0, DH]])
                bvc = su2[:, 1924:2180]
                bvb = bass.AP(tensor=bvc.tensor, offset=bvc.offset,
                              ap=[bvc.ap[0], [0, NSUB], [DH, H], [1, DH]])
                numf = outp.tile([128, NSUB, D], f32, tag="numf")
                nc.vector.tensor_tensor(
                    out=numf.rearrange("p s (h d) -> p s h d", h=H),
                    in0=gcb, in1=bvb, op=OP.mult)
                nc.vector.tensor_add(numf, numf, psA[:, :, 0:256])
            # per-s pipeline: msg (DVE) -> transpose (PE) -> copy (scalar)
            # -> Wo + residual (PE) -> bn stats (DVE)
            msg = outp.tile([128, NSUB, D], bf, tag="msg")
            pst = pvk.tile([128, 2 * NSUB, 128], bf, tag="vk", name="pst")
            msgT = outp.tile([128, 2 * NSUB, 128], bf, tag="msgT")
            ps_o = accp.tile([128, NSUB, 512], f32, tag="A", name="pso")
            stats = small.tile([128, NSUB, 6], f32, tag="stats")
            mv = small.tile([128, NSUB, 2], f32, tag="mv")
            if numf is not None:
                num_in = numf.rearrange("p s (h d) -> p s h d", h=H)
            else:
                num_in = psA[:, :, 0:256].rearrange(
                    "p s (h d) -> p s h d", h=H)
            rdb = bass.AP(tensor=rden.tensor, offset=rden.offset,
                          ap=[rden.ap[0], rden.ap[1], rden.ap[2], [0, DH]])
            nc.vector.tensor_tensor(
                out=msg.rearrange("p s (h d) -> p s h d", h=H),
                in0=num_in, in1=rdb, op=OP.mult)
            for s in range(NSUB):
                for b in range(2):
                    nc.tensor.transpose(pst[:, 2 * s + b, :],
                                        msg[:, s, ts(b, 128)], ident)
                nc.scalar.activation(msgT[:, 2 * s:2 * s + 2, :],
                                     pst[:, 2 * s:2 * s + 2, :], AF.Copy)
                nc.tensor.matmul(ps_o[:, s, 0:256], msgT[:, 2 * s, :],
                                 WoT_sb[:, 0, :], start=True, stop=False)
                nc.tensor.matmul(ps_o[:, s, 0:256], msgT[:, 2 * s + 1, :],
                                 WoT_sb[:, 1, :], start=False, stop=False)
                # x = msg@WoT + (h + bo): residual added on the PE
                nc.tensor.matmul(ps_o[:, s, 0:256], ident, hs_sb[:, s, :],
                                 start=False, stop=True)
                nc.vector.bn_stats(out=stats[:, s, :], in_=ps_o[:, s, 0:256])
                nc.vector.bn_aggr(out=mv[:, s, :], in_=stats[:, s, :])

            sd = small.tile([128, NSUB], f32, tag="sd")
            nc.scalar.activation(sd, mv[:, :, 1], AF.Sqrt, bias=eps_sb)
            rstd = small.tile([128, NSUB], f32, tag="rstd")
            nc.vector.reciprocal(rstd, sd)
            # y = x*rstd - mean*rstd, done on the scalar engine as
            # Identity(x*scale + bias) with per-partition scale/bias APs
            nmr = small.tile([128, NSUB], f32, tag="nmr")
            nc.vector.tensor_tensor(out=nmr, in0=mv[:, :, 0], in1=rstd,
                                    op=OP.mult)
            nc.vector.tensor_scalar(nmr, nmr, -1.0, None, op0=OP.mult)

            ot = outp.tile([128, NSUB, D], f32, tag="ot")
            if has_gb:
                y = outp.tile([128, NSUB, D], bf, tag="y")
                for s in range(NSUB):
                    nc.scalar.activation(y[:, s, :], ps_o[:, s, 0:256],
                                         AF.Identity, bias=nmr[:, s:s + 1],
                                         scale=rstd[:, s:s + 1])
                gamb = bass.AP(tensor=gam.tensor, offset=gam.offset,
                               ap=[gam.ap[0], [0, NSUB], gam.ap[1]])
                betb = bass.AP(tensor=bet.tensor, offset=bet.offset,
                               ap=[bet.ap[0], [0, NSUB], bet.ap[1]])
                yg = outp.tile([128, NSUB, D], bf, tag="yg")
                nc.vector.tensor_tensor(out=yg, in0=y, in1=gamb, op=OP.mult)
                nc.vector.tensor_tensor(out=ot, in0=yg, in1=betb, op=OP.add)
                nc.sync.dma_start(out_d.ap(), ot)
            else:
                # gamma==1, beta==0: normalized x is the final output
                for s in range(NSUB):
                    nc.scalar.activation(ot[:, s, :], ps_o[:, s, 0:256],
                                         AF.Identity, bias=nmr[:, s:s + 1],
                                         scale=rstd[:, s:s + 1])
                    nc.sync.dma_start(out_d.ap()[:, s, :], ot[:, s, :])

    nc.compile()
    return nc


def _make_in_maps(h, w, Wk, Wv, bv, We_w, u, Wo, bo, gamma, beta, **_unused):
    import ml_dtypes
    f = np.float32
    b16 = ml_dtypes.bfloat16
    e4 = ml_dtypes.float8_e4m3
    h = np.asarray(h, dtype=f)
    w = np.asarray(w, dtype=f)
    Wk = np.asarray(Wk, dtype=f)
    u = np.asarray(u, dtype=f)
    We_w = np.asarray(We_w, dtype=f)
    bv = np.asarray(bv, dtype=f)
    gamma_f = np.asarray(gamma, dtype=f)
    beta_f = np.asarray(beta, dtype=f)
    has_bv = bool(np.any(bv != 0))
    has_gb = bool(np.any(gamma_f != 1.0) or np.any(beta_f != 0.0))

    # ht[p, a, j] = h[j, a*128+p]
    ht = np.ascontiguousarray(
        h.T.reshape(2, 128, N).transpose(1, 0, 2)).astype(e4)

    # su1: Wv^T blocks + a_k blocks
    su1 = np.zeros((128, 2, 260), f)
    for a in range(2):
        su1[:, a, 0:256] = np.asarray(Wv, dtype=f)[:, a * 128:(a + 1) * 128].T
    ak = np.einsum('hdc,hd->ch', Wk.reshape(H, DH, D), u[:, DH:2 * DH])
    su1[:, 0, 256:260] = ak[0:128, :]
    su1[:, 1, 256:260] = ak[128:256, :]

    # su2: WoT | identity | hs (per core) | c1 | gamma | beta | bv
    c1 = np.einsum('hd,hd->h', We_w[:, 0].reshape(H, DE),
                   u[:, 2 * DH:2 * DH + DE])
    su2_base = np.zeros((128, 2180), f)
    WoT = np.asarray(Wo, dtype=f).T
    su2_base[:, 0:512] = WoT.reshape(2, 128, D).transpose(
        1, 0, 2).reshape(128, 512)
    su2_base[:, 512:640] = np.eye(128, dtype=f)
    su2_base[:, 1408:1412] = c1[None, :]
    su2_base[:, 1412:1668] = gamma_f[None, :]
    su2_base[:, 1668:1924] = beta_f[None, :]
    if has_bv:
        su2_base[:, 1924:2180] = bv[None, :]

    bo_f = np.asarray(bo, dtype=f)
    wT_relu = np.maximum(w.T, 0.0)

    common = {
        "ht": ht,
        "su1": su1.astype(b16),
    }
    in_maps = []
    for c in range(NCORES):
        sl = slice(c * ISLICE, (c + 1) * ISLICE)
        m = dict(common)
        # wt[p, ch, 0, jm, ii] = relu(w)[i0+ii, (ch*4+jm)*128+p]
        # wt[p, ch, 1, jm, ii] = mask
        wtc = wT_relu[:, sl].reshape(NCH, CJT, 128, ISLICE)
        wtc = wtc.transpose(2, 0, 1, 3).reshape(128, NCH, 1, CHW)
        mskc = (wtc > 0).astype(f)
        m["wt"] = np.ascontiguousarray(
            np.concatenate([wtc, mskc], axis=2).reshape(
                128, NCH * 2 * CHW)).astype(e4)
        su2 = su2_base.copy()
        su2[:, 640:1408] = (h[sl, :] + bo_f[None, :]).reshape(
            NSUB, 128, D).transpose(1, 0, 2).reshape(128, 768)
        m["su2"] = su2.astype(b16)
        in_maps.append(m)
    return in_maps, (has_bv, has_gb)


def kernel(**inputs):
    from concourse.bass_utils import run_bass_kernel_spmd

    in_maps, flags = _make_in_maps(**inputs)
    key = ("nc",) + flags
    if key not in _cache:
        _cache[key] = _build_bass(flags)
    nc = _cache[key]

    res = run_bass_kernel_spmd(nc, in_maps, core_ids=list(range(NCORES)))
    parts = [np.asarray(r["out"]).transpose(1, 0, 2).reshape(ISLICE, D)
             for r in res.results]
    out = np.concatenate(parts, axis=0)
    return np.ascontiguousarray(out, dtype=np.float32)
